# revision 36
# baseline (speedup 1.0000x reference)
"""Deformable 3D conv (offset-predicting conv + trilinear-sampled 3x3x3 deform conv)
on 8 TRN2 NeuronCores.

Strategy: shard the output D axis (4 planes/core). Per core:
  1. Offset conv as 27 shifted fp16 matmuls (PE, row-packed 4x K=32).
  2. p-pipeline on DVE: clip, floor, fracs, gather indices.
  3. Build a 2x2x2-corner block table in DRAM (one 512B row per padded voxel,
     all 8 trilinear corners x 32 channels, fp16) via xbar DMA transposes.
  4. Indirect-DMA gather: ONE instruction per 2-chunk group ([128, 54] offset
     AP -> 6912 descriptors) instead of per (chunk, n) calls.
  5. Trilinear reduction as 8 precomputed corner weights (fp16 stored as
     duplicated pairs to keep DVE 2x packing): 1 mult + 3 tree adds.
  6. Contraction over (n, c) as 7 accumulated PE matmuls with N=512.
"""
import os
import sys

for _p in ('/opt/trn_rl_repo', '/root/.axon_site/_ro/trn_rl_repo'):
    if os.path.isdir(_p) and _p not in sys.path:
        sys.path.insert(0, _p)

import numpy as np
import ml_dtypes  # noqa

import concourse.bass as bass
import concourse.mybir as mybir
import concourse.tile as tile
from concourse import bacc
from concourse import library_config
from concourse.bass_utils import run_bass_kernel_spmd
from concourse.masks import make_identity

F32 = mybir.dt.float32
F16 = mybir.dt.float16
I32 = mybir.dt.int32
I16 = mybir.dt.int16
AL = mybir.AluOpType

# ---------------- problem constants ----------------
C = 32          # input channels
O = 64          # output channels
NN = 27         # kernel sample points
NCORES = 8
DSH = 4         # d-planes per core
V = DSH * 32 * 32   # voxels per core = 4096
P35 = 35
PL = 16         # XE d-planes per core
PLSZ = P35 * P35    # 1225
XE_ROWS = PL * PLSZ  # 19600
TROWS = 19712        # 154 * 128 (padded table rows)
GRPS = TROWS // 128  # 154
XE_FREE = 22400      # >= TROWS + max shift (1191) = 20903; keep 128-mult slack
NVT = 8              # conv v-tiles of 512
NVC = 32             # combine v-chunks of 128
NG2 = 16             # 2-chunk gather/lerp groups
NG4 = 8              # 4-chunk contraction groups

_PROGRAM_CACHE = {}
DEBUG = False


def _build_program():
    nc = bacc.Bacc("TRN2", target_bir_lowering=False, debug=False)

    xe_d = nc.dram_tensor("xe", [C, XE_FREE], F16, kind="ExternalInput").ap()
    pc_d = nc.dram_tensor("pc", [128, NVC * 96], F32, kind="ExternalInput").ap()
    dclip_d = nc.dram_tensor("dclip", [128, 2], F32, kind="ExternalInput").ap()
    wp_d = nc.dram_tensor("wp", [96, 9 * 96], F16, kind="ExternalInput").ap()
    wd_d = nc.dram_tensor("wd", [128, 7 * O], F16, kind="ExternalInput").ap()
    out_d = nc.dram_tensor("out_sl", [O, V], F32, kind="ExternalOutput").ap()
    t_d = nc.dram_tensor("tdram", [TROWS, 256], F16).ap()
    if DEBUG:
        dbg_idx = nc.dram_tensor("dbg_idx", [128, NVC * 27], I32,
                                 kind="ExternalOutput").ap()
        dbg_i16 = nc.dram_tensor("dbg_i16", [128, 7168], I16,
                                 kind="ExternalOutput").ap()
        dbg_frac = nc.dram_tensor("dbg_frac", [128, NVC * 96], F16,
                                  kind="ExternalOutput").ap()
        dbg_wtc = nc.dram_tensor("dbg_wtc", [128, NVC * 27 * 16], F16,
                                 kind="ExternalOutput").ap()
        dbg_rt = nc.dram_tensor("dbg_rt", [128, 54 * 256], F16,
                                kind="ExternalOutput").ap()
        dbg_acc8 = nc.dram_tensor("dbg_acc8", [128, 54 * 256], F16,
                                  kind="ExternalOutput").ap()
        dbg_acc4c = nc.dram_tensor("dbg_acc4c", [128, 4 * 896], F16,
                                   kind="ExternalOutput").ap()

    with tile.TileContext(nc) as tc:
        with tc.tile_pool(name="const", bufs=1) as cpool:
            ident = cpool.tile([128, 128], F32)
            make_identity(nc, ident[:])
            wp_sb = cpool.tile([96, 9 * 96], F16)
            nc.sync.dma_start(wp_sb[:], wp_d)
            wd_sb = cpool.tile([128, 7 * O], F16)
            nc.sync.dma_start(wd_sb[:], wd_d)
            pc_sb = cpool.tile([128, NVC * 96], F32)
            nc.sync.dma_start(pc_sb[:], pc_d)
            dclip_sb = cpool.tile([128, 2], F32)
            nc.sync.dma_start(dclip_sb[:], dclip_d)

            # kept for whole kernel: fp16 frac, int16 gather idxs, corner wts
            frac_t = cpool.tile([128, NVC * 96], F16)
            # idx16[q, n2*8 + t] = table row for voxel p = t*16+q, sample n2
            # (dma_gather wrapped layout, replicated across 16-row groups)
            idx16 = cpool.tile([128, 896 * 8], I16)
            # wtc: col = ((vc*27+n)*8 + e)*2 + dup   (e = ed*4+eh*2+ew)
            wtc = cpool.tile([128, NVC * 27 * 16], F16)

            with tc.tile_pool(name="xe", bufs=1) as xepool:
                # rows 32k..32k+31 hold xe shifted left by k (w-tap packing)
                xe4 = xepool.tile([96, XE_FREE], F16)
                nc.sync.dma_start(xe4[0:32, :], xe_d)
                nc.sync.dma_start(xe4[32:64, 0:XE_FREE - 1], xe_d[:, 1:])
                nc.sync.dma_start(xe4[64:96, 0:XE_FREE - 2], xe_d[:, 2:])

                # ---------- phase 2: corner-block table ----------
                with tc.tile_pool(name="tbl", bufs=2) as tpool:
                    for half in range(2):
                        t_sb = tpool.tile([128, 77 * 256], F16, tag="tsb")
                        base = half * 77 * 128
                        for ed in range(2):
                            for eh in range(2):
                                for ew in range(2):
                                    e = ed * 4 + eh * 2 + ew
                                    dlt = ed * PLSZ + eh * P35 + ew
                                    src = xe4[0:32, base + dlt: base + dlt + 77 * 128]
                                    dst = t_sb[:].rearrange(
                                        "p (g x) -> p g x", x=256
                                    )[:, :, e * 32:(e + 1) * 32]
                                    nc.sync.dma_start_transpose(out=dst, in_=src)
                        # DRAM layout: row' = p*GRPS + g  (contiguous per partition)
                        dst_ap = t_d.rearrange("(p G) x -> p G x", G=GRPS)[
                            :, half * 77:(half + 1) * 77, :]
                        nc.sync.dma_start(
                            out=dst_ap,
                            in_=t_sb[:].rearrange("p (g x) -> p g x", x=256))

                # ---------- phase 3: offset conv ----------
                # one matmul per (kd, kh): K=96 (3 w-taps packed), N=4096
                off_sb = xepool.tile([96, NVT * 512], F32)
                with tc.tile_pool(name="convps", bufs=1, space="PSUM") as cps:
                    pscs = [cps.tile([96, 512], F32, tag=f"cv{vt}",
                                     name=f"cv{vt}")
                            for vt in range(NVT)]
                    for kk in range(9):
                        kd, kh = kk // 3, kk % 3
                        for vt in range(NVT):
                            dl, hh = vt // 2, vt % 2
                            b0 = (dl + kd + 5) * PLSZ + (hh * 16 + kh) * P35
                            rhs = xe4[:, b0:b0 + 16 * P35].rearrange(
                                "p (a b) -> p a b", b=P35)[:, :, 0:32]
                            nc.tensor.matmul(
                                pscs[vt][:, :],
                                lhsT=wp_sb[:, kk * 96:(kk + 1) * 96],
                                rhs=rhs,
                                start=(kk == 0),
                                stop=(kk == 8),
                            )
                    for vt in range(NVT):
                        nc.scalar.copy(
                            off_sb[:, vt * 512:(vt + 1) * 512], pscs[vt][:, :])
                with (
                    tc.tile_pool(name="trps", bufs=2, space="PSUM") as tps,
                    tc.tile_pool(name="pipe2", bufs=1) as pipe,
                ):

                    # transpose [96, 128]-chunks -> [128, 96] and add pc
                    # (4 transposes per psum bank, one add per group)
                    p_t = pipe.tile([128, NVC * 96], F32)
                    p_tv = p_t[:].rearrange("p (g x) -> p g x", x=96)
                    pc_v = pc_sb[:].rearrange("p (g x) -> p g x", x=96)
                    for cg in range(8):
                        ptp = tps.tile([128, 4, 96], F32, tag="trps")
                        for j in range(4):
                            ch = cg * 4 + j
                            nc.tensor.transpose(
                                ptp[:, j, :],
                                off_sb[:, ch * 128:(ch + 1) * 128],
                                ident[0:96, 0:96],
                            )
                        nc.vector.tensor_add(
                            p_tv[:, cg * 4:(cg + 1) * 4, :], ptp[:],
                            pc_v[:, cg * 4:(cg + 1) * 4, :])

                    # ---------- p-pipeline ----------
                    dv = p_t[:].rearrange("p (v x) -> p v x", x=96)[:, :, 0:27]
                    hwv = p_t[:].rearrange("p (v x) -> p v x", x=96)[:, :, 32:91]
                    # d-axis clip to [lo, hi] (per-core values via dclip)
                    nc.vector.scalar_tensor_tensor(
                        out=dv, in0=dv, scalar=dclip_sb[:, 0:1],
                        in1=dclip_sb[:, 1:2].rearrange(
                            "p (a b) -> p a b", b=1).to_broadcast((128, NVC, 27)),
                        op0=AL.max, op1=AL.min)
                    # h/w clip to [0, 33] (includes junk cols, harmless)
                    nc.vector.tensor_scalar(
                        out=hwv, in0=hwv, scalar1=0.0, scalar2=33.0,
                        op0=AL.max, op1=AL.min)

                    q0i = pipe.tile([128, NVC * 96], I32)
                    nc.vector.tensor_copy(q0i[:], p_t[:])
                    q0f = pipe.tile([128, NVC * 96], F32)
                    nc.vector.tensor_copy(q0f[:], q0i[:])
                    # guard against round-to-nearest casts: q0f -= (q0f > p)
                    fixt = pipe.tile([128, NVC * 96], F32)
                    nc.vector.tensor_tensor(out=fixt[:], in0=q0f[:], in1=p_t[:],
                                            op=AL.is_gt)
                    nc.vector.tensor_sub(q0f[:], q0f[:], fixt[:])
                    # frac (fp16)
                    nc.vector.tensor_sub(frac_t[:], p_t[:], q0f[:])
                    # d-axis safety clamp to [0, 14]
                    q0dv = q0f[:].rearrange("p (v x) -> p v x", x=96)[:, :, 0:27]
                    nc.vector.tensor_scalar(
                        out=q0dv, in0=q0dv, scalar1=0.0, scalar2=14.0,
                        op0=AL.max, op1=AL.min)

                    # idx = (q0d*1225 + q0h)*35 + q0w  (row-major local row r)
                    q0hv = q0f[:].rearrange("p (v x) -> p v x", x=96)[:, :, 32:59]
                    q0wv = q0f[:].rearrange("p (v x) -> p v x", x=96)[:, :, 64:91]
                    idxf = pipe.tile([128, 896], F32)
                    nc.vector.memset(idxf[:, 864:896], 0.0)
                    ia = idxf[:, 0:NVC * 27]
                    iv = ia.rearrange("p (v x) -> p v x", x=27)
                    nc.vector.scalar_tensor_tensor(
                        out=iv, in0=q0dv, scalar=35.0, in1=q0hv,
                        op0=AL.mult, op1=AL.add)
                    nc.vector.scalar_tensor_tensor(
                        out=iv, in0=iv, scalar=35.0, in1=q0wv,
                        op0=AL.mult, op1=AL.add)
                    # remap r -> r' = (r % 128)*GRPS + (r // 128)
                    rg = pipe.tile([128, NVC * 27], F32)
                    nc.vector.tensor_scalar_mul(rg[:], ia, 1.0 / 128.0)
                    rgi = pipe.tile([128, NVC * 27], I32)
                    nc.vector.tensor_copy(rgi[:], rg[:])
                    rgf = pipe.tile([128, NVC * 27], F32)
                    nc.vector.tensor_copy(rgf[:], rgi[:])
                    fix2 = pipe.tile([128, NVC * 27], F32)
                    nc.vector.tensor_tensor(out=fix2[:], in0=rgf[:], in1=rg[:],
                                            op=AL.is_gt)
                    nc.vector.tensor_sub(rgf[:], rgf[:], fix2[:])
                    # rp = r - 128*g ; r' = rp*GRPS + g
                    nc.vector.scalar_tensor_tensor(
                        out=ia, in0=rgf[:], scalar=-128.0, in1=ia,
                        op0=AL.mult, op1=AL.add)
                    nc.vector.scalar_tensor_tensor(
                        out=ia, in0=ia, scalar=float(GRPS), in1=rgf[:],
                        op0=AL.mult, op1=AL.add)
                    if DEBUG:
                        idx32d = pipe.tile([128, NVC * 27], I32)
                        nc.vector.tensor_copy(idx32d[:], ia)
                        nc.sync.dma_start(dbg_idx, idx32d[:])

                    # ---- build wrapped int16 idx tile for dma_gather ----
                    # idx16[q, (n2*8+t)] = idxf[t*16+q, n2]
                    with (
                        tc.tile_pool(name="t1ps", bufs=2, space="PSUM") as t1ps,
                        tc.tile_pool(name="t2ps", bufs=2, space="PSUM") as t2ps,
                        tc.tile_pool(name="tsbp", bufs=1) as tsbp,
                    ):
                        tsb = tsbp.tile([128, 896], F32)
                        for cb in range(7):
                            pT = t1ps.tile([128, 128], F32, tag="t1")
                            nc.tensor.transpose(
                                pT[:, :], idxf[:, cb * 128:(cb + 1) * 128],
                                ident[:, :])
                            nc.scalar.copy(
                                tsb[:, cb * 128:(cb + 1) * 128], pT[:, :])
                        i16v = idx16[:].rearrange(
                            "p (n t) -> p n t", t=8)
                        for cb in range(7):
                            p2 = t2ps.tile([16, 8, 128], F32, tag="t2")
                            for t in range(8):
                                nc.tensor.transpose(
                                    p2[:, t, :],
                                    tsb[:, cb * 128 + t * 16:
                                        cb * 128 + (t + 1) * 16],
                                    ident[:, :])
                            nc.vector.tensor_copy(
                                i16v[0:16, cb * 128:(cb + 1) * 128, :],
                                p2[:].rearrange("q t n -> q n t"))
                            # replicate rows 0:16 across the other 7 groups
                            # (per cb-block, so early gathers start sooner)
                            for k in range(1, 8):
                                nc.sync.dma_start(
                                    out=idx16[16 * k:16 * (k + 1),
                                              cb * 1024:(cb + 1) * 1024],
                                    in_=idx16[0:16,
                                              cb * 1024:(cb + 1) * 1024])

                # ---------- phase 3.5: corner weights (pairs layout) ----------
                with tc.tile_pool(name="wprep", bufs=1) as wpool:
                    A = NVC * 27  # 864
                    fd2 = wpool.tile([128, A * 2], F16)
                    fh2 = wpool.tile([128, A * 2], F16)
                    fw2 = wpool.tile([128, A * 2], F16)
                    gd2 = wpool.tile([128, A * 2], F16)
                    gh2 = wpool.tile([128, A * 2], F16)
                    gw2 = wpool.tile([128, A * 2], F16)
                    ph = wpool.tile([128, A * 8], F16)  # col = a*8 + ehw*2 + dup

                    frac_v = frac_t[:].rearrange("p (v x) -> p v x", x=96)
                    for f2, g2t, col0 in ((fd2, gd2, 0), (fh2, gh2, 32),
                                          (fw2, gw2, 64)):
                        src = frac_v[:, :, col0:col0 + 27].to_broadcast(
                            (128, NVC, 27, 2))
                        nc.vector.tensor_copy(
                            f2[:].rearrange("p (v n d) -> p v n d", n=27, d=2),
                            src)
                        # g = 1 - f
                        nc.vector.tensor_scalar(
                            out=g2t[:], in0=f2[:], scalar1=-1.0, scalar2=1.0,
                            op0=AL.mult, op1=AL.add)

                    ph_v = ph[:].rearrange("p (a e d) -> p a e d", e=4, d=2)
                    for ehw, (hh_t, ww_t) in enumerate(
                            ((gh2, gw2), (gh2, fw2), (fh2, gw2), (fh2, fw2))):
                        nc.vector.tensor_tensor(
                            out=ph_v[:, :, ehw, :],
                            in0=hh_t[:].rearrange("p (a d) -> p a d", d=2),
                            in1=ww_t[:].rearrange("p (a d) -> p a d", d=2),
                            op=AL.mult)

                    wtc_v = wtc[:].rearrange("p (a e d) -> p a e d", e=8, d=2)
                    for ed, dd_t in enumerate((gd2, fd2)):
                        dd_v = dd_t[:].rearrange("p (a d) -> p a d", d=2)
                        for ehw in range(4):
                            nc.vector.tensor_tensor(
                                out=wtc_v[:, :, ed * 4 + ehw, :],
                                in0=ph_v[:, :, ehw, :],
                                in1=dd_v,
                                op=AL.mult)

            # ---------- phase 4: gather + corner-reduce + contract ----------
            with (
                tc.tile_pool(name="gat", bufs=2) as gpool,
                tc.tile_pool(name="lrp", bufs=1) as lpool,
                tc.tile_pool(name="accc", bufs=2) as apool,
                tc.tile_pool(name="tr", bufs=2) as tpool2,
                tc.tile_pool(name="ops", bufs=2, space="PSUM") as ops,
                tc.tile_pool(name="outp", bufs=2) as opool,
            ):
                for g4 in range(NG4):
                    acc4c = apool.tile([128, 4 * 896], F16, tag="acc4c")
                    nc.vector.memset(
                        acc4c[:].rearrange("p (c x) -> p c x", x=896)
                        [:, :, 864:896], 0.0)
                    for h2 in range(2):
                        g2 = g4 * 2 + h2
                        rt2 = gpool.tile([128, 54 * 256], F16, tag="rt")
                        rt2v = rt2[:].rearrange("p (g x) -> p g x", x=256)
                        for n0 in range(0, 54, 8):
                            k = min(8, 54 - n0)
                            c0 = (g2 * 54 + n0) * 8
                            nc.gpsimd.dma_gather(
                                out_ap=rt2v[:, n0:n0 + k, :],
                                in_ap=t_d,
                                idxs_ap=idx16[:, c0:c0 + k * 8],
                                num_idxs=k * 128,
                                num_idxs_reg=k * 128,
                                elem_size=256,
                            )
                        # weighted corners: acc8 = rt2 * wtc (2x-packed pairs)
                        acc8 = lpool.tile([128, 54 * 256], F16, tag="acc8")
                        nc.vector.tensor_tensor(
                            out=acc8[:].rearrange(
                                "p (a s d) -> p a s d", s=16, d=2),
                            in0=rt2[:].rearrange(
                                "p (a s d) -> p a s d", s=16, d=2),
                            in1=wtc[:, g2 * 864:(g2 + 1) * 864].rearrange(
                                "p (a d) -> p a d", d=2
                            ).to_broadcast((128, 432, 2, 16)).rearrange(
                                "p a d s -> p a s d"),
                            op=AL.mult)
                        # tree-add the 8 corners: 8 -> 4 -> 2 -> 1
                        acc4t = lpool.tile([128, 54 * 128], F16, tag="acc4")
                        a8 = acc8[:].rearrange(
                            "p (a e c) -> p a e c", e=8, c=32)
                        nc.vector.tensor_add(
                            acc4t[:].rearrange("p (a e c) -> p a e c",
                                               e=4, c=32),
                            a8[:, :, 0:4, :], a8[:, :, 4:8, :])
                        acc2t = lpool.tile([128, 54 * 64], F16, tag="acc2")
                        a4 = acc4t[:].rearrange(
                            "p (a e c) -> p a e c", e=4, c=32)
                        nc.vector.tensor_add(
                            acc2t[:].rearrange("p (a e c) -> p a e c",
                                               e=2, c=32),
                            a4[:, :, 0:2, :], a4[:, :, 2:4, :])
                        a2 = acc2t[:].rearrange(
                            "p (l n e c) -> p l n e c", l=2, n=27, e=2, c=32)
                        out_v = acc4c[:].rearrange(
                            "p (cc n c) -> p cc n c", cc=4, n=28, c=32)
                        nc.vector.tensor_add(
                            out_v[:, h2 * 2:h2 * 2 + 2, 0:27, :],
                            a2[:, :, :, 0, :], a2[:, :, :, 1, :])
                        if DEBUG and g2 == 0:
                            nc.sync.dma_start(dbg_rt, rt2[:, :])
                            nc.sync.dma_start(dbg_acc8, acc8[:, :])

                    # transpose + contract + write out
                    acct = tpool2.tile([128, 28, 128], F16, tag="acct")
                    nc.sync.dma_start_transpose(
                        out=acct[:], in_=acc4c[:])
                    pso = ops.tile([64, 512], F32, tag="pso")
                    pso_v = pso[:].rearrange("p (a b) -> p a b", b=128)
                    acct_v = acct[:].rearrange("p (a b) c -> p a b c", b=7)
                    for g in range(7):
                        nc.tensor.matmul(
                            pso_v,
                            lhsT=wd_sb[:, g * O:(g + 1) * O],
                            rhs=acct_v[:, :, g, :],
                            start=(g == 0), stop=(g == 6))
                    osb = opool.tile([64, 512], F32, tag="osb")
                    nc.scalar.copy(osb[:], pso[:, :])
                    nc.sync.dma_start(
                        out=out_d[:, g4 * 512:(g4 + 1) * 512], in_=osb[:])
                    if DEBUG and g4 == 0:
                        nc.sync.dma_start(dbg_acc4c, acc4c[:, :])
                if DEBUG:
                    nc.sync.dma_start(dbg_i16, idx16[:])
                    nc.sync.dma_start(dbg_frac, frac_t[:])
                    nc.sync.dma_start(dbg_wtc, wtc[:])

    nc.compile()
    return nc


def _host_prep(x, w_p, b_p, w_d):
    """Build per-core input maps."""
    x = np.asarray(x, np.float32)
    w_p = np.asarray(w_p, np.float32)
    b_p = np.asarray(b_p, np.float32)
    w_d = np.asarray(w_d, np.float32)

    # global padded/extended volume, channel-first, fp16:
    # XG[c, g, h', w'] with g = xp_plane + 5 (xp planes -5..39), h', w' in [0,35)
    XG = np.zeros((C, 45, P35, P35), np.float16)
    XG[:, 6:38, 1:33, 1:33] = x[0].astype(np.float16)

    # pc (shared): [128, 32*96] f32
    v = np.arange(V)
    dl, hh, wl = v >> 10, (v >> 5) & 31, v & 31
    r = np.array([-1.0, 0.0, 1.0], np.float32)
    pn_d, pn_h, pn_w = np.meshgrid(r, r, r, indexing='ij')
    pn = np.stack([pn_d.ravel(), pn_h.ravel(), pn_w.ravel()])  # (3, 27)
    pc = np.zeros((V, 96), np.float32)
    pc[:, 0:27] = (dl[:, None] + 6.0) + pn[0][None, :] + b_p[None, 0:27]
    pc[:, 32:59] = (hh[:, None] + 1.0) + pn[1][None, :] + b_p[None, 27:54]
    pc[:, 64:91] = (wl[:, None] + 1.0) + pn[2][None, :] + b_p[None, 54:81]
    pc_t = pc.reshape(NVC, 128, 96).transpose(1, 0, 2).reshape(128, NVC * 96)
    pc_t = np.ascontiguousarray(pc_t, np.float32)

    # wp lhsT: [96, 9*96] fp16; K-row kw*32+cin, col block (kd*3+kh)*96
    wp_l = np.zeros((96, 9 * 96), np.float16)
    colmap = np.full(96, -1, np.int64)
    colmap[0:27] = np.arange(27)
    colmap[32:59] = 27 + np.arange(27)
    colmap[64:91] = 54 + np.arange(27)
    for kk in range(9):
        kd, kh = kk // 3, kk % 3
        for kw in range(3):
            for m in range(96):
                ch = colmap[m]
                if ch < 0:
                    continue
                wp_l[kw * 32:(kw + 1) * 32, kk * 96 + m] = \
                    w_p[ch, :, kd, kh, kw]

    # wd lhsT: [128, 7*64] fp16 (K-row (g, pk): n = 4g + pk//32, c = pk%32)
    wd_l = np.zeros((128, 7 * O), np.float16)
    for g in range(7):
        for pk in range(128):
            n = 4 * g + pk // 32
            if n >= NN:
                continue
            wd_l[pk, g * O:(g + 1) * O] = w_d[:, pk % 32, n // 9, (n // 3) % 3, n % 3]

    in_maps = []
    for k in range(NCORES):
        dlo = 4 * k - 5
        xe = np.zeros((C, XE_FREE), np.float16)
        xe[:, :XE_ROWS] = XG[:, 4 * k:4 * k + PL].reshape(C, XE_ROWS)
        dclip = np.zeros((128, 2), np.float32)
        dclip[:, 0] = 0.0 - dlo
        dclip[:, 1] = 33.0 - dlo
        in_maps.append({
            "xe": xe,
            "pc": pc_t,
            "dclip": dclip,
            "wp": wp_l,
            "wd": wd_l,
        })
    return in_maps


def kernel(x, w_p, b_p, w_d):
    if "nc" not in _PROGRAM_CACHE:
        _PROGRAM_CACHE["nc"] = _build_program()
    nc = _PROGRAM_CACHE["nc"]
    in_maps = _host_prep(x, w_p, b_p, w_d)
    res = run_bass_kernel_spmd(nc, in_maps, list(range(NCORES))).results
    out = np.empty((1, O, 32, 32, 32), np.float32)
    for k in range(NCORES):
        out[0, :, 4 * k:4 * k + 4] = res[k]["out_sl"].reshape(O, DSH, 32, 32)
    return out


# revision 48
# speedup vs baseline: 1.2191x; 1.2191x over previous
"""Deformable 3D conv (offset-predicting conv + trilinear-sampled 3x3x3 deform conv)
on 8 TRN2 NeuronCores.

Strategy: shard the output D axis (4 planes/core). Per core:
  1. Offset conv as 27 shifted fp16 matmuls (PE, row-packed 4x K=32).
  2. p-pipeline on DVE: clip, floor, fracs, gather indices.
  3. Build a 2x2x2-corner block table in DRAM (one 512B row per padded voxel,
     all 8 trilinear corners x 32 channels, fp16) via xbar DMA transposes.
  4. Indirect-DMA gather: ONE instruction per 2-chunk group ([128, 54] offset
     AP -> 6912 descriptors) instead of per (chunk, n) calls.
  5. Trilinear reduction as 8 precomputed corner weights (fp16 stored as
     duplicated pairs to keep DVE 2x packing): 1 mult + 3 tree adds.
  6. Contraction over (n, c) as 7 accumulated PE matmuls with N=512.
"""
import os
import sys

for _p in ('/opt/trn_rl_repo', '/root/.axon_site/_ro/trn_rl_repo'):
    if os.path.isdir(_p) and _p not in sys.path:
        sys.path.insert(0, _p)

import numpy as np
import ml_dtypes  # noqa

import concourse.bass as bass
import concourse.mybir as mybir
import concourse.tile as tile
from concourse import bacc
from concourse import library_config
from concourse.bass_utils import run_bass_kernel_spmd
from concourse.masks import make_identity

F32 = mybir.dt.float32
F16 = mybir.dt.float16
I32 = mybir.dt.int32
I16 = mybir.dt.int16
AL = mybir.AluOpType

# ---------------- problem constants ----------------
C = 32          # input channels
O = 64          # output channels
NN = 27         # kernel sample points
NCORES = 8
DSH = 4         # d-planes per core
V = DSH * 32 * 32   # voxels per core = 4096
P35 = 35
PL = 16         # XE d-planes per core
PLSZ = P35 * P35    # 1225
XE_ROWS = PL * PLSZ  # 19600
TROWS = 19712        # 154 * 128 (padded table rows)
GRPS = TROWS // 128  # 154
XE_FREE = 22400      # >= TROWS + max shift (1191) = 20903; keep 128-mult slack
NVT = 8              # conv v-tiles of 512
NVC = 32             # combine v-chunks of 128
NG2 = 16             # 2-chunk gather/lerp groups
NG4 = 8              # 4-chunk contraction groups

_PROGRAM_CACHE = {}
DEBUG = False


def _build_program():
    nc = bacc.Bacc("TRN2", target_bir_lowering=False, debug=False)

    xe_d = nc.dram_tensor("xe", [C, XE_FREE], F16, kind="ExternalInput").ap()
    pc_d = nc.dram_tensor("pc", [128, NVC * 96], F32, kind="ExternalInput").ap()
    dclip_d = nc.dram_tensor("dclip", [128, 2], F32, kind="ExternalInput").ap()
    wp_d = nc.dram_tensor("wp", [96, 9 * 96], F16, kind="ExternalInput").ap()
    wd_d = nc.dram_tensor("wd", [128, 7 * O], F16, kind="ExternalInput").ap()
    out_d = nc.dram_tensor("out_sl", [O, V], F32, kind="ExternalOutput").ap()
    t_d = nc.dram_tensor("tdram", [TROWS, 256], F16).ap()
    if DEBUG:
        dbg_idx = nc.dram_tensor("dbg_idx", [128, NVC * 27], I32,
                                 kind="ExternalOutput").ap()
        dbg_i16 = nc.dram_tensor("dbg_i16", [128, 7168], I16,
                                 kind="ExternalOutput").ap()
        dbg_frac = nc.dram_tensor("dbg_frac", [128, NVC * 96], F16,
                                  kind="ExternalOutput").ap()
        dbg_wtc = nc.dram_tensor("dbg_wtc", [128, NVC * 27 * 16], F16,
                                 kind="ExternalOutput").ap()
        dbg_rt = nc.dram_tensor("dbg_rt", [128, 54 * 256], F16,
                                kind="ExternalOutput").ap()
        dbg_acc8 = nc.dram_tensor("dbg_acc8", [128, 54 * 256], F16,
                                  kind="ExternalOutput").ap()
        dbg_acc4c = nc.dram_tensor("dbg_acc4c", [128, 4 * 896], F16,
                                   kind="ExternalOutput").ap()

    with tile.TileContext(nc) as tc:
        with tc.tile_pool(name="const", bufs=1) as cpool:
            ident = cpool.tile([128, 128], F32)
            make_identity(nc, ident[:])
            wp_sb = cpool.tile([96, 9 * 96], F16)
            nc.sync.dma_start(wp_sb[:], wp_d)
            wd_sb = cpool.tile([128, 7 * O], F16)
            nc.sync.dma_start(wd_sb[:], wd_d)
            pc_sb = cpool.tile([128, NVC * 96], F32)
            nc.sync.dma_start(pc_sb[:], pc_d)
            dclip_sb = cpool.tile([128, 2], F32)
            nc.sync.dma_start(dclip_sb[:], dclip_d)

            # kept for whole kernel: fp16 frac, int16 gather idxs, corner wts
            frac_t = cpool.tile([128, NVC * 96], F16)
            # idx16[q, n2*8 + t] = table row for voxel p = t*16+q, sample n2
            # (dma_gather wrapped layout, replicated across 16-row groups)
            idx16 = cpool.tile([128, 896 * 8], I16)
            # wtc: col = ((vc*27+n)*8 + e)*2 + dup   (e = ed*4+eh*2+ew)
            wtc = cpool.tile([128, NVC * 27 * 16], F16)

            with tc.tile_pool(name="xe", bufs=1) as xepool:
                # rows 32k..32k+31 hold xe shifted left by k (w-tap packing)
                xe4 = xepool.tile([96, XE_FREE], F16)
                nc.sync.dma_start(xe4[0:32, :], xe_d)
                nc.sync.dma_start(xe4[32:64, 0:XE_FREE - 1], xe_d[:, 1:])
                nc.sync.dma_start(xe4[64:96, 0:XE_FREE - 2], xe_d[:, 2:])
                # p-pipe tiles allocated BEFORE the tbl pool opens: keeps them
                # off the tbl pool's address range, so p-pipe writes don't
                # inherit a WAR dependency on the (slow) table stores
                off_sb = xepool.tile([96, NVT * 512], F32)
                p_t = xepool.tile([128, NVC * 96], F32)
                q0i = xepool.tile([128, NVC * 96], I32)
                q0f = xepool.tile([128, NVC * 96], F32)
                idxf = xepool.tile([128, 896], F32)

                # ---------- phase 2: corner-block table ----------
                with tc.tile_pool(name="tbl", bufs=1) as tpool:
                    for half in range(2):
                        t_sb = tpool.tile([128, 77 * 256], F16, tag="tsb")
                        base = half * 77 * 128
                        for ed in range(2):
                            for eh in range(2):
                                for ew in range(2):
                                    e = ed * 4 + eh * 2 + ew
                                    dlt = ed * PLSZ + eh * P35 + ew
                                    src = xe4[0:32, base + dlt: base + dlt + 77 * 128]
                                    dst = t_sb[:].rearrange(
                                        "p (g x) -> p g x", x=256
                                    )[:, :, e * 32:(e + 1) * 32]
                                    nc.sync.dma_start_transpose(out=dst, in_=src)
                        # DRAM layout: row' = p*GRPS + g  (contiguous per partition)
                        dst_ap = t_d.rearrange("(p G) x -> p G x", G=GRPS)[
                            :, half * 77:(half + 1) * 77, :]
                        nc.sync.dma_start(
                            out=dst_ap,
                            in_=t_sb[:].rearrange("p (g x) -> p g x", x=256))

                # ---------- phase 3: offset conv ----------
                # one matmul per (kd, kh): K=96 (3 w-taps packed), N=4096
                with tc.tile_pool(name="convps", bufs=1, space="PSUM") as cps:
                    pscs = [cps.tile([96, 512], F32, tag=f"cv{vt}",
                                     name=f"cv{vt}")
                            for vt in range(NVT)]
                    for kk in range(9):
                        kd, kh = kk // 3, kk % 3
                        for vt in range(NVT):
                            dl, hh = vt // 2, vt % 2
                            b0 = (dl + kd + 5) * PLSZ + (hh * 16 + kh) * P35
                            rhs = xe4[:, b0:b0 + 16 * P35].rearrange(
                                "p (a b) -> p a b", b=P35)[:, :, 0:32]
                            nc.tensor.matmul(
                                pscs[vt][:, :],
                                lhsT=wp_sb[:, kk * 96:(kk + 1) * 96],
                                rhs=rhs,
                                start=(kk == 0),
                                stop=(kk == 8),
                            )
                    for vt in range(NVT):
                        nc.scalar.copy(
                            off_sb[:, vt * 512:(vt + 1) * 512], pscs[vt][:, :])
                with (
                    tc.tile_pool(name="trps", bufs=2, space="PSUM") as tps,
                    tc.tile_pool(name="pipe2", bufs=1) as pipe,
                ):

                    # transpose [96, 128]-chunks -> [128, 96] and add pc
                    for ch in range(NVC):
                        ptp = tps.tile([128, 96], F32, tag="trps")
                        nc.tensor.transpose(
                            ptp[:, :],
                            off_sb[:, ch * 128:(ch + 1) * 128],
                            ident[0:96, 0:96],
                        )
                        nc.vector.tensor_add(
                            p_t[:, ch * 96:(ch + 1) * 96], ptp[:, :],
                            pc_sb[:, ch * 96:(ch + 1) * 96])

                    # ---------- p-pipeline ----------
                    dv = p_t[:].rearrange("p (v x) -> p v x", x=96)[:, :, 0:27]
                    hwv = p_t[:].rearrange("p (v x) -> p v x", x=96)[:, :, 32:91]
                    # d-axis clip to [lo, hi] (per-core values via dclip)
                    nc.vector.scalar_tensor_tensor(
                        out=dv, in0=dv, scalar=dclip_sb[:, 0:1],
                        in1=dclip_sb[:, 1:2].rearrange(
                            "p (a b) -> p a b", b=1).to_broadcast((128, NVC, 27)),
                        op0=AL.max, op1=AL.min)
                    # h/w clip to [0, 33] (includes junk cols, harmless)
                    nc.vector.tensor_scalar(
                        out=hwv, in0=hwv, scalar1=0.0, scalar2=33.0,
                        op0=AL.max, op1=AL.min)

                    nc.vector.tensor_copy(q0i[:], p_t[:])
                    nc.vector.tensor_copy(q0f[:], q0i[:])
                    # guard against round-to-nearest casts: q0f -= (q0f > p)
                    # (fixt reuses q0i's buffer; q0i is dead after the copy)
                    fixt = q0i[:].bitcast(F32)
                    nc.vector.tensor_tensor(out=fixt, in0=q0f[:], in1=p_t[:],
                                            op=AL.is_gt)
                    nc.vector.tensor_sub(q0f[:], q0f[:], fixt)
                    # frac (fp16)
                    nc.vector.tensor_sub(frac_t[:], p_t[:], q0f[:])
                    # d-axis safety clamp to [0, 14]
                    q0dv = q0f[:].rearrange("p (v x) -> p v x", x=96)[:, :, 0:27]
                    nc.vector.tensor_scalar(
                        out=q0dv, in0=q0dv, scalar1=0.0, scalar2=14.0,
                        op0=AL.max, op1=AL.min)

                    # idx = (q0d*1225 + q0h)*35 + q0w  (row-major local row r)
                    q0hv = q0f[:].rearrange("p (v x) -> p v x", x=96)[:, :, 32:59]
                    q0wv = q0f[:].rearrange("p (v x) -> p v x", x=96)[:, :, 64:91]
                    nc.vector.memset(idxf[:, 864:896], 0.0)
                    ia = idxf[:, 0:NVC * 27]
                    iv = ia.rearrange("p (v x) -> p v x", x=27)
                    nc.vector.scalar_tensor_tensor(
                        out=iv, in0=q0dv, scalar=35.0, in1=q0hv,
                        op0=AL.mult, op1=AL.add)
                    nc.vector.scalar_tensor_tensor(
                        out=iv, in0=iv, scalar=35.0, in1=q0wv,
                        op0=AL.mult, op1=AL.add)
                    # remap r -> r' = (r % 128)*GRPS + (r // 128)
                    rg = pipe.tile([128, NVC * 27], F32)
                    nc.vector.tensor_scalar_mul(rg[:], ia, 1.0 / 128.0)
                    rgi = pipe.tile([128, NVC * 27], I32)
                    nc.vector.tensor_copy(rgi[:], rg[:])
                    rgf = pipe.tile([128, NVC * 27], F32)
                    nc.vector.tensor_copy(rgf[:], rgi[:])
                    fix2 = pipe.tile([128, NVC * 27], F32)
                    nc.vector.tensor_tensor(out=fix2[:], in0=rgf[:], in1=rg[:],
                                            op=AL.is_gt)
                    nc.vector.tensor_sub(rgf[:], rgf[:], fix2[:])
                    # rp = r - 128*g ; r' = rp*GRPS + g
                    nc.vector.scalar_tensor_tensor(
                        out=ia, in0=rgf[:], scalar=-128.0, in1=ia,
                        op0=AL.mult, op1=AL.add)
                    nc.vector.scalar_tensor_tensor(
                        out=ia, in0=ia, scalar=float(GRPS), in1=rgf[:],
                        op0=AL.mult, op1=AL.add)
                    if DEBUG:
                        idx32d = pipe.tile([128, NVC * 27], I32)
                        nc.vector.tensor_copy(idx32d[:], ia)
                        nc.sync.dma_start(dbg_idx, idx32d[:])

                    # ---- build wrapped int16 idx tile for dma_gather ----
                    # idx16[q, (n2*8+t)] = idxf[t*16+q, n2]
                    with (
                        tc.tile_pool(name="t1ps", bufs=2, space="PSUM") as t1ps,
                        tc.tile_pool(name="t2ps", bufs=2, space="PSUM") as t2ps,
                        tc.tile_pool(name="tsbp", bufs=1) as tsbp,
                    ):
                        tsb = tsbp.tile([128, 896], F32)
                        for cb in range(7):
                            pT = t1ps.tile([128, 128], F32, tag="t1")
                            nc.tensor.transpose(
                                pT[:, :], idxf[:, cb * 128:(cb + 1) * 128],
                                ident[:, :])
                            nc.scalar.copy(
                                tsb[:, cb * 128:(cb + 1) * 128], pT[:, :])
                        i16v = idx16[:].rearrange(
                            "p (n t) -> p n t", t=8)
                        for cb in range(7):
                            p2 = t2ps.tile([16, 8, 128], F32, tag="t2")
                            for t in range(8):
                                nc.tensor.transpose(
                                    p2[:, t, :],
                                    tsb[:, cb * 128 + t * 16:
                                        cb * 128 + (t + 1) * 16],
                                    ident[:, :])
                            nc.vector.tensor_copy(
                                i16v[0:16, cb * 128:(cb + 1) * 128, :],
                                p2[:].rearrange("q t n -> q n t"))
                            # replicate rows 0:16 across the other 7 groups
                            # (per cb-block, so early gathers start sooner)
                            for k in range(1, 8):
                                nc.sync.dma_start(
                                    out=idx16[16 * k:16 * (k + 1),
                                              cb * 1024:(cb + 1) * 1024],
                                    in_=idx16[0:16,
                                              cb * 1024:(cb + 1) * 1024])

                # ---------- phase 3.5: corner weights (pairs layout) ----------
                with tc.tile_pool(name="wprep", bufs=1) as wpool:
                    A = NVC * 27  # 864
                    fd2 = wpool.tile([128, A * 2], F16)
                    fh2 = wpool.tile([128, A * 2], F16)
                    fw2 = wpool.tile([128, A * 2], F16)
                    gd2 = wpool.tile([128, A * 2], F16)
                    gh2 = wpool.tile([128, A * 2], F16)
                    gw2 = wpool.tile([128, A * 2], F16)
                    ph = wpool.tile([128, A * 8], F16)  # col = a*8 + ehw*2 + dup

                    frac_v = frac_t[:].rearrange("p (v x) -> p v x", x=96)
                    for f2, g2t, col0 in ((fd2, gd2, 0), (fh2, gh2, 32),
                                          (fw2, gw2, 64)):
                        src = frac_v[:, :, col0:col0 + 27].to_broadcast(
                            (128, NVC, 27, 2))
                        nc.vector.tensor_copy(
                            f2[:].rearrange("p (v n d) -> p v n d", n=27, d=2),
                            src)
                        # g = 1 - f
                        nc.vector.tensor_scalar(
                            out=g2t[:], in0=f2[:], scalar1=-1.0, scalar2=1.0,
                            op0=AL.mult, op1=AL.add)

                    ph_v = ph[:].rearrange("p (a e d) -> p a e d", e=4, d=2)
                    for ehw, (hh_t, ww_t) in enumerate(
                            ((gh2, gw2), (gh2, fw2), (fh2, gw2), (fh2, fw2))):
                        nc.vector.tensor_tensor(
                            out=ph_v[:, :, ehw, :],
                            in0=hh_t[:].rearrange("p (a d) -> p a d", d=2),
                            in1=ww_t[:].rearrange("p (a d) -> p a d", d=2),
                            op=AL.mult)

                    wtc_v = wtc[:].rearrange("p (a e d) -> p a e d", e=8, d=2)
                    for ed, dd_t in enumerate((gd2, fd2)):
                        dd_v = dd_t[:].rearrange("p (a d) -> p a d", d=2)
                        for ehw in range(4):
                            nc.vector.tensor_tensor(
                                out=wtc_v[:, :, ed * 4 + ehw, :],
                                in0=ph_v[:, :, ehw, :],
                                in1=dd_v,
                                op=AL.mult)

            # ---------- phase 4: gather + corner-reduce + contract ----------
            with (
                tc.tile_pool(name="gat", bufs=2) as gpool,
                tc.tile_pool(name="lrp", bufs=1) as lpool,
                tc.tile_pool(name="accc", bufs=2) as apool,
                tc.tile_pool(name="tr", bufs=2) as tpool2,
                tc.tile_pool(name="ops", bufs=2, space="PSUM") as ops,
                tc.tile_pool(name="outp", bufs=2) as opool,
            ):
                for g4 in range(NG4):
                    acc4c = apool.tile([128, 4 * 896], F16, tag="acc4c")
                    nc.vector.memset(
                        acc4c[:].rearrange("p (c x) -> p c x", x=896)
                        [:, :, 864:896], 0.0)
                    for h2 in range(2):
                        g2 = g4 * 2 + h2
                        rt2 = gpool.tile([128, 54 * 256], F16, tag="rt")
                        rt2v = rt2[:].rearrange("p (g x) -> p g x", x=256)
                        for n0 in range(0, 54, 8):
                            k = min(8, 54 - n0)
                            c0 = (g2 * 54 + n0) * 8
                            nc.gpsimd.dma_gather(
                                out_ap=rt2v[:, n0:n0 + k, :],
                                in_ap=t_d,
                                idxs_ap=idx16[:, c0:c0 + k * 8],
                                num_idxs=k * 128,
                                num_idxs_reg=k * 128,
                                elem_size=256,
                            )
                        # weighted corners: acc8 = rt2 * wtc (2x-packed pairs)
                        acc8 = lpool.tile([128, 54 * 256], F16, tag="acc8")
                        nc.vector.tensor_tensor(
                            out=acc8[:].rearrange(
                                "p (a s d) -> p a s d", s=16, d=2),
                            in0=rt2[:].rearrange(
                                "p (a s d) -> p a s d", s=16, d=2),
                            in1=wtc[:, g2 * 864:(g2 + 1) * 864].rearrange(
                                "p (a d) -> p a d", d=2
                            ).to_broadcast((128, 432, 2, 16)).rearrange(
                                "p a d s -> p a s d"),
                            op=AL.mult)
                        # tree-add the 8 corners: 8 -> 4 -> 2 -> 1
                        acc4t = lpool.tile([128, 54 * 128], F16, tag="acc4")
                        a8 = acc8[:].rearrange(
                            "p (a e c) -> p a e c", e=8, c=32)
                        nc.vector.tensor_add(
                            acc4t[:].rearrange("p (a e c) -> p a e c",
                                               e=4, c=32),
                            a8[:, :, 0:4, :], a8[:, :, 4:8, :])
                        acc2t = lpool.tile([128, 54 * 64], F16, tag="acc2")
                        a4 = acc4t[:].rearrange(
                            "p (a e c) -> p a e c", e=4, c=32)
                        nc.vector.tensor_add(
                            acc2t[:].rearrange("p (a e c) -> p a e c",
                                               e=2, c=32),
                            a4[:, :, 0:2, :], a4[:, :, 2:4, :])
                        a2 = acc2t[:].rearrange(
                            "p (l n e c) -> p l n e c", l=2, n=27, e=2, c=32)
                        out_v = acc4c[:].rearrange(
                            "p (cc n c) -> p cc n c", cc=4, n=28, c=32)
                        nc.vector.tensor_add(
                            out_v[:, h2 * 2:h2 * 2 + 2, 0:27, :],
                            a2[:, :, :, 0, :], a2[:, :, :, 1, :])
                        if DEBUG and g2 == 0:
                            nc.sync.dma_start(dbg_rt, rt2[:, :])
                            nc.sync.dma_start(dbg_acc8, acc8[:, :])

                    # transpose + contract + write out
                    acct = tpool2.tile([128, 28, 128], F16, tag="acct")
                    nc.sync.dma_start_transpose(
                        out=acct[:], in_=acc4c[:])
                    pso = ops.tile([64, 512], F32, tag="pso")
                    pso_v = pso[:].rearrange("p (a b) -> p a b", b=128)
                    acct_v = acct[:].rearrange("p (a b) c -> p a b c", b=7)
                    for g in range(7):
                        nc.tensor.matmul(
                            pso_v,
                            lhsT=wd_sb[:, g * O:(g + 1) * O],
                            rhs=acct_v[:, :, g, :],
                            start=(g == 0), stop=(g == 6))
                    osb = opool.tile([64, 512], F32, tag="osb")
                    nc.scalar.copy(osb[:], pso[:, :])
                    nc.sync.dma_start(
                        out=out_d[:, g4 * 512:(g4 + 1) * 512], in_=osb[:])
                    if DEBUG and g4 == 0:
                        nc.sync.dma_start(dbg_acc4c, acc4c[:, :])
                if DEBUG:
                    nc.sync.dma_start(dbg_i16, idx16[:])
                    nc.sync.dma_start(dbg_frac, frac_t[:])
                    nc.sync.dma_start(dbg_wtc, wtc[:])

    nc.compile()
    return nc


def _host_prep(x, w_p, b_p, w_d):
    """Build per-core input maps."""
    x = np.asarray(x, np.float32)
    w_p = np.asarray(w_p, np.float32)
    b_p = np.asarray(b_p, np.float32)
    w_d = np.asarray(w_d, np.float32)

    # global padded/extended volume, channel-first, fp16:
    # XG[c, g, h', w'] with g = xp_plane + 5 (xp planes -5..39), h', w' in [0,35)
    XG = np.zeros((C, 45, P35, P35), np.float16)
    XG[:, 6:38, 1:33, 1:33] = x[0].astype(np.float16)

    # pc (shared): [128, 32*96] f32
    v = np.arange(V)
    dl, hh, wl = v >> 10, (v >> 5) & 31, v & 31
    r = np.array([-1.0, 0.0, 1.0], np.float32)
    pn_d, pn_h, pn_w = np.meshgrid(r, r, r, indexing='ij')
    pn = np.stack([pn_d.ravel(), pn_h.ravel(), pn_w.ravel()])  # (3, 27)
    pc = np.zeros((V, 96), np.float32)
    pc[:, 0:27] = (dl[:, None] + 6.0) + pn[0][None, :] + b_p[None, 0:27]
    pc[:, 32:59] = (hh[:, None] + 1.0) + pn[1][None, :] + b_p[None, 27:54]
    pc[:, 64:91] = (wl[:, None] + 1.0) + pn[2][None, :] + b_p[None, 54:81]
    pc_t = pc.reshape(NVC, 128, 96).transpose(1, 0, 2).reshape(128, NVC * 96)
    pc_t = np.ascontiguousarray(pc_t, np.float32)

    # wp lhsT: [96, 9*96] fp16; K-row kw*32+cin, col block (kd*3+kh)*96
    wp_l = np.zeros((96, 9 * 96), np.float16)
    colmap = np.full(96, -1, np.int64)
    colmap[0:27] = np.arange(27)
    colmap[32:59] = 27 + np.arange(27)
    colmap[64:91] = 54 + np.arange(27)
    for kk in range(9):
        kd, kh = kk // 3, kk % 3
        for kw in range(3):
            for m in range(96):
                ch = colmap[m]
                if ch < 0:
                    continue
                wp_l[kw * 32:(kw + 1) * 32, kk * 96 + m] = \
                    w_p[ch, :, kd, kh, kw]

    # wd lhsT: [128, 7*64] fp16 (K-row (g, pk): n = 4g + pk//32, c = pk%32)
    wd_l = np.zeros((128, 7 * O), np.float16)
    for g in range(7):
        for pk in range(128):
            n = 4 * g + pk // 32
            if n >= NN:
                continue
            wd_l[pk, g * O:(g + 1) * O] = w_d[:, pk % 32, n // 9, (n // 3) % 3, n % 3]

    in_maps = []
    for k in range(NCORES):
        dlo = 4 * k - 5
        xe = np.zeros((C, XE_FREE), np.float16)
        xe[:, :XE_ROWS] = XG[:, 4 * k:4 * k + PL].reshape(C, XE_ROWS)
        dclip = np.zeros((128, 2), np.float32)
        dclip[:, 0] = 0.0 - dlo
        dclip[:, 1] = 33.0 - dlo
        in_maps.append({
            "xe": xe,
            "pc": pc_t,
            "dclip": dclip,
            "wp": wp_l,
            "wd": wd_l,
        })
    return in_maps


def kernel(x, w_p, b_p, w_d):
    if "nc" not in _PROGRAM_CACHE:
        _PROGRAM_CACHE["nc"] = _build_program()
    nc = _PROGRAM_CACHE["nc"]
    in_maps = _host_prep(x, w_p, b_p, w_d)
    res = run_bass_kernel_spmd(nc, in_maps, list(range(NCORES))).results
    out = np.empty((1, O, 32, 32, 32), np.float32)
    for k in range(NCORES):
        out[0, :, 4 * k:4 * k + 4] = res[k]["out_sl"].reshape(O, DSH, 32, 32)
    return out
